# revision 2
# baseline (speedup 1.0000x reference)
"""Routed-LoRA linear layer (moe_routing) on 8 trn2 NeuronCores.

Math (per token t):
  out[t, :] = W @ x[t] + b + 2.0 * sum_n mask[n, t] * (B_n @ (A_n @ x[t]))

Strategy:
  - Data-parallel over B*T = 65536 tokens: 8192 tokens per core.
  - Streaming operands (x, W, A, B, mask) are marshaled to bf16 host-side:
    halves HBM traffic and SBUF footprint; worst-case output error ~2e-3
    relative, well inside the 2e-2 gate. PSUM accumulation stays fp32.
  - All operand transposes are done host-side so the device only ever
    streams contiguous, partition-friendly layouts:
      xt  [D_IN, TOK]  = x-shard transposed      (contraction dim major)
      wt  [D_IN, D_OUT] = W.T
      at  [D_IN, NR]    = fused-A.T
      btr [NR, D_OUT]   = fused-B.T
      msk [NR, TOK]     = routing mask expanded to rank dim, pre-scaled
  - LoRA delta is accumulated into the same PSUM bank as the base matmul
    (the fused-B matmul is just a 9th contraction chunk); bias is added
    during the PSUM->SBUF copy; output is stored bf16 and upcast on host.
  - First supertile's x is DMA'd per contraction chunk so the PE starts
    ~2.5us in instead of waiting for a monolithic load; output DMAs are
    per-128-token tile so the drain tail is short.
"""

import numpy as np
import ml_dtypes

import concourse.bass as bass
from concourse import bacc
import concourse.mybir as mybir
import concourse.tile as tile
from concourse.bass_utils import run_bass_kernel_spmd

N_CORES = 8
B, T = 8, 8192
D_IN = 1024
D_OUT = 1024
N_ADAPT, R = 4, 16
NR = N_ADAPT * R  # 64
SCALING = 32.0 / 16.0

TOK = B * T // N_CORES  # 8192 tokens per core
SUP = 512               # tokens per supertile
N_SUP = TOK // SUP      # 16
SUB = 128               # tokens per matmul M-tile
N_SUB = SUP // SUB      # 4
P = 128
KC = D_IN // P          # 8 contraction chunks
NB = D_OUT // 512       # 2 PSUM-bank column halves

F32 = mybir.dt.float32
BF16 = mybir.dt.bfloat16
NP_BF16 = ml_dtypes.bfloat16


def build_bass(xp_bufs=4, op_bufs=3, pso_bufs=3):
    nc = bacc.Bacc(
        "TRN2", target_bir_lowering=False, debug=False, num_devices=N_CORES
    )

    xt_d = nc.dram_tensor("xt", [D_IN, TOK], BF16, kind="ExternalInput")
    wt_d = nc.dram_tensor("wt", [D_IN, D_OUT], BF16, kind="ExternalInput")
    at_d = nc.dram_tensor("at", [D_IN, NR], BF16, kind="ExternalInput")
    bt_d = nc.dram_tensor("btr", [NR, D_OUT], BF16, kind="ExternalInput")
    bias_d = nc.dram_tensor("bias", [D_OUT], F32, kind="ExternalInput")
    msk_d = nc.dram_tensor("msk", [NR, TOK], BF16, kind="ExternalInput")
    out_d = nc.dram_tensor("out", [TOK, D_OUT], BF16, kind="ExternalOutput")

    xt_r = xt_d.ap().rearrange("(kc p) t -> p kc t", p=P)
    wt_r = wt_d.ap().rearrange("(kc p) n -> p kc n", p=P)
    at_r = at_d.ap().rearrange("(kc p) j -> p kc j", p=P)
    out_r = out_d.ap().rearrange("(s q p) n -> s q p n", q=N_SUB, p=P)
    bias_bcast = bass.AP(
        tensor=bias_d, offset=0, ap=[[0, P], [1, D_OUT]]
    )

    with tile.TileContext(nc) as tc:
        with (
            tc.tile_pool(name="const", bufs=1) as const,
            tc.tile_pool(name="xp", bufs=xp_bufs) as xp,
            tc.tile_pool(name="sp", bufs=2) as sp,
            tc.tile_pool(name="op", bufs=op_bufs) as op,
            tc.tile_pool(name="pss", bufs=2, space="PSUM") as pss,
            tc.tile_pool(name="pso", bufs=pso_bufs, space="PSUM") as pso,
        ):
            w_sb = const.tile([P, KC, D_OUT], BF16)
            a_sb = const.tile([P, KC, NR], BF16)
            bt_sb = const.tile([NR, D_OUT], BF16)
            b_sb = const.tile([P, D_OUT], F32)
            m_sb = const.tile([NR, TOK], BF16)
            # Preload order matters for startup latency: the first s-pass
            # matmuls need a_sb + x0 chunk 0 (sync queue); the first main
            # matmuls need bt + W chunk k in order (scalar queue).
            nc.sync.dma_start(out=a_sb[:], in_=at_r)
            nc.scalar.dma_start(out=bt_sb[:], in_=bt_d.ap())
            for k in range(KC):
                nc.scalar.dma_start(out=w_sb[:, k, :], in_=wt_r[:, k, :])
            nc.gpsimd.dma_start(out=b_sb[:], in_=bias_bcast)

            for s in range(N_SUP):
                t0 = s * SUP
                x_sb = xp.tile([P, KC, SUP], BF16, tag="x")
                if s == 0:
                    # chunked first load: A-pass matmul k can start as soon
                    # as chunk k lands instead of after the full supertile
                    for k in range(KC):
                        nc.sync.dma_start(
                            out=x_sb[:, k, :], in_=xt_r[:, k, t0 : t0 + SUP]
                        )
                else:
                    nc.sync.dma_start(
                        out=x_sb[:], in_=xt_r[:, :, t0 : t0 + SUP]
                    )
                nc.sync.dma_start(
                    out=m_sb[:, t0 : t0 + SUP],
                    in_=msk_d.ap()[:, t0 : t0 + SUP],
                )

                # s.T = fused_A @ x.T for this supertile: [NR, SUP]
                s_ps = pss.tile([NR, SUP], F32, tag="sps")
                for k in range(KC):
                    nc.tensor.matmul(
                        s_ps[:],
                        a_sb[:, k, :],
                        x_sb[:, k, :],
                        start=(k == 0),
                        stop=(k == KC - 1),
                    )
                sm_sb = sp.tile([NR, SUP], BF16, tag="sm")
                nc.vector.tensor_mul(
                    sm_sb[:], s_ps[:], m_sb[:, t0 : t0 + SUP]
                )

                for q in range(N_SUB):
                    ts = q * SUB
                    o_ps = pso.tile([P, D_OUT], F32, tag="ops")
                    for n in range(NB):
                        nsl = slice(n * 512, (n + 1) * 512)
                        for k in range(KC):
                            nc.tensor.matmul(
                                o_ps[:, nsl],
                                x_sb[:, k, ts : ts + SUB],
                                w_sb[:, k, nsl],
                                start=(k == 0),
                                stop=False,
                            )
                        nc.tensor.matmul(
                            o_ps[:, nsl],
                            sm_sb[:, ts : ts + SUB],
                            bt_sb[:, nsl],
                            start=False,
                            stop=True,
                        )
                    o_sb = op.tile([P, D_OUT], BF16, tag="o")
                    nc.vector.tensor_add(o_sb[:], o_ps[:], b_sb[:])
                    nc.scalar.dma_start(out=out_r[s, q], in_=o_sb[:])

    nc.compile()
    return nc


_NC_CACHE = None


def _get_nc():
    global _NC_CACHE
    if _NC_CACHE is None:
        _NC_CACHE = build_bass()
    return _NC_CACHE


def make_in_maps(x, W, b, lora_A, lora_B, masks):
    x = np.ascontiguousarray(x, dtype=np.float32)
    W = np.ascontiguousarray(W, dtype=np.float32)
    b = np.ascontiguousarray(b, dtype=np.float32)
    lora_A = np.ascontiguousarray(lora_A, dtype=np.float32)
    lora_B = np.ascontiguousarray(lora_B, dtype=np.float32)
    masks = np.ascontiguousarray(masks, dtype=np.float32)

    x_flat = x.reshape(B * T, D_IN)
    A_flat = lora_A.reshape(NR, D_IN)
    B_flat = lora_B.transpose(1, 0, 2).reshape(D_OUT, NR)

    wt = np.ascontiguousarray(W.T.astype(NP_BF16))       # [D_IN, D_OUT]
    at = np.ascontiguousarray(A_flat.T.astype(NP_BF16))  # [D_IN, NR]
    btr = np.ascontiguousarray(B_flat.T.astype(NP_BF16))  # [NR, D_OUT]

    m_full = masks[..., 0].reshape(N_ADAPT, B * T) * np.float32(SCALING)
    m_exp = np.repeat(m_full, R, axis=0).astype(NP_BF16)  # [NR, B*T]
    xt_full = x_flat.astype(NP_BF16)

    in_maps = []
    for c in range(N_CORES):
        sl = slice(c * TOK, (c + 1) * TOK)
        in_maps.append(
            {
                "xt": np.ascontiguousarray(xt_full[sl].T),
                "wt": wt,
                "at": at,
                "btr": btr,
                "bias": b,
                "msk": np.ascontiguousarray(m_exp[:, sl]),
            }
        )
    return in_maps


def kernel(x, W, b, lora_A, lora_B, masks):
    nc = _get_nc()
    in_maps = make_in_maps(x, W, b, lora_A, lora_B, masks)
    res = run_bass_kernel_spmd(nc, in_maps, core_ids=list(range(N_CORES)))
    out = np.concatenate([r["out"] for r in res.results], axis=0)
    out = out.astype(np.float32).reshape(B, T, D_OUT)
    return out


# revision 4
# speedup vs baseline: 1.0031x; 1.0031x over previous
"""Routed-LoRA linear layer (moe_routing) on 8 trn2 NeuronCores.

Math (per token t):
  out[t, :] = W @ x[t] + b + 2.0 * sum_n mask[n, t] * (B_n @ (A_n @ x[t]))

Strategy:
  - Data-parallel over B*T = 65536 tokens: 8192 tokens per core.
  - Streaming operands (x, W, A, B, mask) are marshaled to bf16 host-side:
    halves HBM traffic and SBUF footprint; worst-case output error ~2e-3
    relative, well inside the 2e-2 gate. PSUM accumulation stays fp32.
  - All operand transposes are done host-side so the device only ever
    streams contiguous, partition-friendly layouts:
      xt  [D_IN, TOK]  = x-shard transposed      (contraction dim major)
      wt  [D_IN, D_OUT] = W.T
      at  [D_IN, NR]    = fused-A.T
      btr [NR, D_OUT]   = fused-B.T
      msk [NR, TOK]     = routing mask expanded to rank dim, pre-scaled
  - LoRA delta is accumulated into the same PSUM bank as the base matmul
    (the fused-B matmul is just a 9th contraction chunk); bias is added
    during the PSUM->SBUF copy; output is stored bf16 and upcast on host.
  - First supertile's x is DMA'd per contraction chunk so the PE starts
    ~2.5us in instead of waiting for a monolithic load; output DMAs are
    per-128-token tile so the drain tail is short.
"""

import numpy as np
import ml_dtypes

import concourse.bass as bass
from concourse import bacc
import concourse.mybir as mybir
import concourse.tile as tile
from concourse.bass_utils import run_bass_kernel_spmd

N_CORES = 8
B, T = 8, 8192
D_IN = 1024
D_OUT = 1024
N_ADAPT, R = 4, 16
NR = N_ADAPT * R  # 64
SCALING = 32.0 / 16.0

TOK = B * T // N_CORES  # 8192 tokens per core
SUP = 512               # tokens per supertile
N_SUP = TOK // SUP      # 16
SUB = 128               # tokens per matmul M-tile
N_SUB = SUP // SUB      # 4
P = 128
KC = D_IN // P          # 8 contraction chunks
NB = D_OUT // 512       # 2 PSUM-bank column halves

F32 = mybir.dt.float32
BF16 = mybir.dt.bfloat16
NP_BF16 = ml_dtypes.bfloat16


def build_bass(xp_bufs=4, op_bufs=4, pso_bufs=6):
    nc = bacc.Bacc(
        "TRN2", target_bir_lowering=False, debug=False, num_devices=N_CORES
    )

    xt_d = nc.dram_tensor("xt", [D_IN, TOK], BF16, kind="ExternalInput")
    wt_d = nc.dram_tensor("wt", [D_IN, D_OUT], BF16, kind="ExternalInput")
    at_d = nc.dram_tensor("at", [D_IN, NR], BF16, kind="ExternalInput")
    bt_d = nc.dram_tensor("btr", [NR, D_OUT], BF16, kind="ExternalInput")
    bias_d = nc.dram_tensor("bias", [D_OUT], F32, kind="ExternalInput")
    msk_d = nc.dram_tensor("msk", [NR, TOK], BF16, kind="ExternalInput")
    out_d = nc.dram_tensor("out", [TOK, D_OUT], BF16, kind="ExternalOutput")

    xt_r = xt_d.ap().rearrange("(kc p) t -> p kc t", p=P)
    wt_r = wt_d.ap().rearrange("(kc p) n -> p kc n", p=P)
    at_r = at_d.ap().rearrange("(kc p) j -> p kc j", p=P)
    out_r = out_d.ap().rearrange("(s q p) n -> s q p n", q=N_SUB, p=P)
    bias_bcast = bass.AP(
        tensor=bias_d, offset=0, ap=[[0, P], [1, D_OUT]]
    )

    with tile.TileContext(nc) as tc:
        with (
            tc.tile_pool(name="const", bufs=1) as const,
            tc.tile_pool(name="xp", bufs=xp_bufs) as xp,
            tc.tile_pool(name="sp", bufs=2) as sp,
            tc.tile_pool(name="op", bufs=op_bufs) as op,
            tc.tile_pool(name="pss", bufs=2, space="PSUM") as pss,
            tc.tile_pool(name="pso", bufs=pso_bufs, space="PSUM") as pso,
        ):
            w_sb = const.tile([P, KC, D_OUT], BF16)
            a_sb = const.tile([P, KC, NR], BF16)
            bt_sb = const.tile([NR, D_OUT], BF16)
            b_sb = const.tile([P, D_OUT], F32)
            m_sb = const.tile([NR, TOK], BF16)
            # Preload order matters for startup latency: the first s-pass
            # matmuls need a_sb + x0 chunk 0 (sync queue); the first main
            # matmuls need bt + W chunk k in order (scalar queue).
            nc.sync.dma_start(out=a_sb[:], in_=at_r)
            nc.scalar.dma_start(out=bt_sb[:], in_=bt_d.ap())
            for k in range(KC):
                nc.scalar.dma_start(out=w_sb[:, k, :], in_=wt_r[:, k, :])
            nc.gpsimd.dma_start(out=b_sb[:], in_=bias_bcast)

            for s in range(N_SUP):
                t0 = s * SUP
                x_sb = xp.tile([P, KC, SUP], BF16, tag="x")
                if s == 0:
                    # chunked first load: consumers of chunk k can start as
                    # soon as chunk k lands instead of after the full load
                    for k in range(KC):
                        nc.sync.dma_start(
                            out=x_sb[:, k, :], in_=xt_r[:, k, t0 : t0 + SUP]
                        )
                else:
                    nc.sync.dma_start(
                        out=x_sb[:], in_=xt_r[:, :, t0 : t0 + SUP]
                    )
                nc.sync.dma_start(
                    out=m_sb[:, t0 : t0 + SUP],
                    in_=msk_d.ap()[:, t0 : t0 + SUP],
                )

                s_ps = pss.tile([NR, SUP], F32, tag="sps")
                sm_sb = sp.tile([NR, SUP], BF16, tag="sm")

                def a_pass(k):
                    # s.T = fused_A @ x.T for this supertile: [NR, SUP]
                    nc.tensor.matmul(
                        s_ps[:],
                        a_sb[:, k, :],
                        x_sb[:, k, :],
                        start=(k == 0),
                        stop=(k == KC - 1),
                        skip_group_check=(s == 0),
                    )

                def mask_mult():
                    nc.vector.tensor_mul(
                        sm_sb[:], s_ps[:], m_sb[:, t0 : t0 + SUP]
                    )

                def main_half(q, n, o_ps_h, interleave_a=False):
                    ts = q * SUB
                    nsl = slice(n * 512, (n + 1) * 512)
                    for k in range(KC):
                        nc.tensor.matmul(
                            o_ps_h[:],
                            x_sb[:, k, ts : ts + SUB],
                            w_sb[:, k, nsl],
                            start=(k == 0),
                            stop=False,
                            skip_group_check=(s == 0),
                        )
                        if interleave_a:
                            a_pass(k)

                def lora_half(q, n, o_ps_h):
                    ts = q * SUB
                    nsl = slice(n * 512, (n + 1) * 512)
                    nc.tensor.matmul(
                        o_ps_h[:],
                        sm_sb[:, ts : ts + SUB],
                        bt_sb[:, nsl],
                        start=False,
                        stop=True,
                        skip_group_check=(s == 0),
                    )
                    o_sb = op.tile([P, 512], BF16, tag="o")
                    nc.vector.tensor_add(o_sb[:], o_ps_h[:], b_sb[:, nsl])
                    nc.scalar.dma_start(
                        out=out_r[s, q][:, nsl], in_=o_sb[:]
                    )

                if s == 0:
                    # Startup schedule: ride the incoming x0 chunk stream.
                    # q0's main matmuls + the A-pass consume chunk k the
                    # moment it lands; the LoRA-B matmuls for each tile are
                    # deferred one tile so they never wait on the mask-mult.
                    tiles = []
                    for q in range(N_SUB):
                        halves = []
                        for n in range(NB):
                            o_ps_h = pso.tile([P, 512], F32, tag="ops")
                            main_half(q, n, o_ps_h,
                                      interleave_a=(q == 0 and n == 0))
                            halves.append((q, n, o_ps_h))
                        if q == 0:
                            mask_mult()
                        tiles.append(halves)
                        if q >= 1:
                            for qq, nn, ph in tiles[q - 1]:
                                lora_half(qq, nn, ph)
                    for qq, nn, ph in tiles[-1]:
                        lora_half(qq, nn, ph)
                else:
                    for k in range(KC):
                        a_pass(k)
                    mask_mult()
                    for q in range(N_SUB):
                        for n in range(NB):
                            o_ps_h = pso.tile([P, 512], F32, tag="ops")
                            main_half(q, n, o_ps_h)
                            lora_half(q, n, o_ps_h)

    nc.compile()
    return nc


_NC_CACHE = None


def _get_nc():
    global _NC_CACHE
    if _NC_CACHE is None:
        _NC_CACHE = build_bass()
    return _NC_CACHE


def make_in_maps(x, W, b, lora_A, lora_B, masks):
    x = np.ascontiguousarray(x, dtype=np.float32)
    W = np.ascontiguousarray(W, dtype=np.float32)
    b = np.ascontiguousarray(b, dtype=np.float32)
    lora_A = np.ascontiguousarray(lora_A, dtype=np.float32)
    lora_B = np.ascontiguousarray(lora_B, dtype=np.float32)
    masks = np.ascontiguousarray(masks, dtype=np.float32)

    x_flat = x.reshape(B * T, D_IN)
    A_flat = lora_A.reshape(NR, D_IN)
    B_flat = lora_B.transpose(1, 0, 2).reshape(D_OUT, NR)

    wt = np.ascontiguousarray(W.T.astype(NP_BF16))       # [D_IN, D_OUT]
    at = np.ascontiguousarray(A_flat.T.astype(NP_BF16))  # [D_IN, NR]
    btr = np.ascontiguousarray(B_flat.T.astype(NP_BF16))  # [NR, D_OUT]

    m_full = masks[..., 0].reshape(N_ADAPT, B * T) * np.float32(SCALING)
    m_exp = np.repeat(m_full, R, axis=0).astype(NP_BF16)  # [NR, B*T]
    xt_full = x_flat.astype(NP_BF16)

    in_maps = []
    for c in range(N_CORES):
        sl = slice(c * TOK, (c + 1) * TOK)
        in_maps.append(
            {
                "xt": np.ascontiguousarray(xt_full[sl].T),
                "wt": wt,
                "at": at,
                "btr": btr,
                "bias": b,
                "msk": np.ascontiguousarray(m_exp[:, sl]),
            }
        )
    return in_maps


def kernel(x, W, b, lora_A, lora_B, masks):
    nc = _get_nc()
    in_maps = make_in_maps(x, W, b, lora_A, lora_B, masks)
    res = run_bass_kernel_spmd(nc, in_maps, core_ids=list(range(N_CORES)))
    out = np.concatenate([r["out"] for r in res.results], axis=0)
    out = out.astype(np.float32).reshape(B, T, D_OUT)
    return out
